# revision 37
# baseline (speedup 1.0000x reference)
"""Trainium2 Bass kernel for nn_LowFreqPenaltyLoss.

Computes mean(|einsum('ih,nchw,jw->ncij', Ch, delta, Cw)|) for
delta [256, 3, 256, 256] f32, Ch/Cw the 8x256 unnormalized DCT-II bases.

Strategy (data-parallel over batch, 8 cores), per core 96 images = 24 MiB:

  LOAD (the problem is HBM-stream-bound): 11 groups of 8 images (2 MiB)
  + tail groups of 4/2/2 images, all via SWDGE DMAs that cast f32->bf16
  inline.  Partition p receives a CONTIGUOUS 16 KiB HBM chunk (group rows
  16p..16p+15), so descriptors/packets are full-sized AND 16 KiB-aligned:
  the old per-row 1 KiB gather streamed at ~275 GB/s read-side, flat
  aligned chunks reach ~400-410 GB/s (17 KiB unaligned strides drop to
  ~310).  All DMAs are issued upfront (all tiles resident, ~100 KiB/part).
  The small tail groups shorten the post-stream serial chain.

  STAGE A (contract h): each partition's rows sit on the free axis, so the
  DCT-H contraction is 16 accumulating matmuls with block-diagonal weights
  wa17[p, r, 8q+i] = Ch[i, h] for global row 16p+r = 256q+h ->
  psumA[8q+i, w] f32 (one [64,256] accumulation per group).

  STAGE B (contract w): ACT copies psumA -> SBUF in two [.,128] halves
  (casts bf16; the halves let the first PE transpose start early), 2 PE
  transposes (each into its own PSUM bank: transpose-mode output must
  start at a bank boundary), DVE copies out, matmul with CwT ->
  ps2[j, (q,i)], fused |.|+sum on DVE into acc[8,1].  Stage B of group g-1
  is emitted AFTER stage A of group g so the PE (which runs in program
  order) never stalls mid-stream on ACT/DVE round-trips.

  FINISH: the tail groups (4/2/2 images) share one tS (free-axis offsets)
  and ONE combined stage-B back-half; acc (main groups) ships early via a
  Sync HWDGE DMA while the tails finish, and the tails' unaccumulated
  reduction ships separately, so the final out-DMA depends only on the last
  DVE reduce.  The host sums 8 cores x 16 partials and divides by 49152.
  bf16 inputs + f32 PSUM accumulation give ~2e-4 relative error (gate 2e-2).

  Known variance: the HBM stream is environment-sensitive -- runs land at
  ~400 GB/s (exec ~76 us), or lower when SDMA engine 15 is slowed by
  descriptor-ring port contention / neighbor-tenant HBM traffic (~85-95
  us).  Engines are partition-pinned, and shedding engine 15's load
  necessarily overloads the rest (+6%), so the uniform aligned layout is
  kept.
"""

import sys
import types

for _p in ("/root/.axon_site/_ro/trn_rl_repo", "/opt/trn_rl_repo"):
    if _p not in sys.path:
        sys.path.append(_p)

import numpy as np
from contextlib import ExitStack

import concourse.bass as bass
import concourse.tile as tile
from concourse import mybir, bass_utils
from concourse._compat import with_exitstack
from concourse.vector_clock import ScopedClock

# ---------------------------------------------------------------------------
# Workarounds for this image.
# ---------------------------------------------------------------------------

# walrus on this image rejects >1 sync-wait on one CTRL instruction; split the
# Tile exit-drain's waits across follow-up nops (same engine, program order).
# Also: the stock tail (barrier + per-sem clear + barrier) costs ~8-10us of
# EVSEM butterfly at kernel end. The kernel is one-shot per NEFF execution and
# NRT re-initialises semaphores per execution, so keep only the drain + DMA
# completion waits.
_ORIG_DAB = tile.TileContext._drain_and_barrier
_USE_STOCK_TAIL = False


def _patched_drain_and_barrier(self, tick_clock, wait_clock):
    if _USE_STOCK_TAIL:
        return _ORIG_DAB(self, tick_clock, wait_clock)
    nc = self.nc
    drain_inst = nc.sync.drain()
    wait_clock.add_sem_waits(
        drain_inst.ins, ScopedClock({None: tick_clock.global_clock})
    )
    si = drain_inst.ins.sync_info
    waits = list(si.on_wait) if si and si.on_wait else []
    if len(waits) > 1:
        drain_inst.ins.sync_info = mybir.SyncInfo(
            on_wait=[waits[0]], on_update=list(si.on_update or [])
        )
        for w in waits[1:]:
            nop = nc.sync.nop(nofuse=True, hint="drain_wait_split")
            nop.ins.sync_info = mybir.SyncInfo(on_wait=[w], on_update=[])
    popped = nc._tile_sem_poison_stack.pop()
    assert popped is self._sem_poison


tile.TileContext._drain_and_barrier = _patched_drain_and_barrier

# zero-egress container: profiling artifact upload must stay local.
bass_utils.upload_artifacts = lambda d: d


def _strip_main_barrier(nc):
    """Drop the prologue all-engine barrier AND the dead const memsets in
    'main': the barrier's only role is to fence the framework preamble (dead
    const memsets + per-engine table loads) from the kernel, but per-engine
    program order already covers the table loads, and nothing reads the
    const tiles (verified: no instruction references const-* memrefs).  Each
    engine then branches into the kernel as soon as its own init finishes
    instead of waiting for the slowest engine (~1.4us of startup)."""
    for fn in nc.m.functions:
        for bb in fn.blocks:
            if bb.name != "main":
                continue
            bb.instructions[:] = [
                i for i in bb.instructions
                if not isinstance(
                    i,
                    (mybir.InstEventSemaphore, mybir.InstDrain, mybir.InstMemset),
                )
            ]


def _split_multi_waits(nc):
    """walrus on this image rejects >1 sync-wait per instruction: hoist extra
    waits onto fresh NoOps inserted just before, on the same engine."""
    for fn in nc.m.functions:
        for bb in fn.blocks:
            new_insts = []
            for inst in bb.instructions:
                si = inst.sync_info
                waits = list(si.on_wait) if si and si.on_wait else []
                if len(waits) > 1:
                    for w in waits[:-1]:
                        nop = mybir.InstNoOp(
                            name=nc.get_next_instruction_name(),
                            sync_info=mybir.SyncInfo(on_wait=[w], on_update=[]),
                            bass_nofuse=True,
                            engine=inst.engine,
                        )
                        new_insts.append(nop)
                    inst.sync_info = mybir.SyncInfo(
                        on_wait=[waits[-1]], on_update=list(si.on_update or [])
                    )
                new_insts.append(inst)
            bb.instructions[:] = new_insts

# ---------------------------------------------------------------------------
# Problem constants (hardcoded; kernel.py must be self-contained).
# ---------------------------------------------------------------------------

B, C, H, W = 256, 3, 256, 256
LOW_A = LOW_B = 8
N_CORES = 8
IMGS_PER_CORE = (B // N_CORES) * C          # 96
GRP = 8                                     # images per main group (2 MiB f32)
N_MAIN = IMGS_PER_CORE // GRP - 1           # 11 main groups; 2x4-img tails
GROWS = GRP * H                             # 2048 rows per main group
SHED_ROWS = 16                              # rows per partition (16 KiB chunks)
TOTAL_LOW = B * C * LOW_A * LOW_B           # 49152 -> mean divisor

F32 = mybir.dt.float32
BF16 = mybir.dt.bfloat16


def _dct_basis(K, N):
    n = np.arange(N, dtype=np.float64)
    k = np.arange(K, dtype=np.float64)
    return (2.0 * np.cos(np.pi * (2.0 * n[None, :] + 1.0) * k[:, None] / (2.0 * N))).astype(
        np.float32
    )


def _make_consts():
    Ch = _dct_basis(LOW_A, H)   # [8, 256]
    Cw = _dct_basis(LOW_B, W)   # [8, 256]
    # Main-group weights: partition p holds group rows 16p..16p+15; global
    # row g = 256q + h -> out row 8q+i gets Ch[i, h] (block-diagonal).
    wa17 = np.zeros((128, SHED_ROWS, 64), np.float32)
    for p in range(128):
        for r in range(SHED_ROWS):
            g = SHED_ROWS * p + r
            q, h = divmod(g, H)
            wa17[p, r, 8 * q:8 * q + 8] = Ch[:, h]
    # 4-image tail group: image q = p//32, rows h = 8*(p%32) + r.
    wa4 = np.zeros((128, 8, 32), np.float32)
    for p in range(128):
        q, pp = p // 32, p % 32
        for r in range(8):
            wa4[p, r, 8 * q:8 * q + 8] = Ch[:, 8 * pp + r]
    # 2-image tail groups: image q = p//64, rows h = 4*(p%64) + r.
    wa2 = np.zeros((128, 4, 16), np.float32)
    for p in range(128):
        q, pp = p // 64, p % 64
        for r in range(4):
            wa2[p, r, 8 * q:8 * q + 8] = Ch[:, 4 * pp + r]
    # cwt[p, wc, j] = Cw[j, wc*128+p]
    cwt = np.zeros((128, 2, LOW_B), np.float32)
    for wc in range(2):
        cwt[:, wc, :] = Cw[:, wc * 128:(wc + 1) * 128].T
    import ml_dtypes
    bf16 = ml_dtypes.bfloat16
    ident = np.eye(128, dtype=np.float32)
    blob = np.concatenate([
        wa17.reshape(128, -1), wa4.reshape(128, -1), wa2.reshape(128, -1),
        cwt.reshape(128, -1), ident,
    ], axis=1)
    return np.ascontiguousarray(blob).astype(bf16)


CONSTS = _make_consts()


# ---------------------------------------------------------------------------
# Kernel body (per core; SPMD over 8 cores).
# ---------------------------------------------------------------------------

@with_exitstack
def _lowfreq_kernel(ctx: ExitStack, tc, out_ap, delta_ap, consts_ap):
    nc = tc.nc

    const_pool = ctx.enter_context(tc.tile_pool(name="const", bufs=1))
    in8_pool = ctx.enter_context(tc.tile_pool(name="in8", bufs=N_MAIN))
    in4_pool = ctx.enter_context(tc.tile_pool(name="in4", bufs=2))
    sS_pool = ctx.enter_context(tc.tile_pool(name="sS", bufs=3))
    tS_pool = ctx.enter_context(tc.tile_pool(name="tS", bufs=3))
    red_pool = ctx.enter_context(tc.tile_pool(name="red", bufs=2))
    acc_pool = ctx.enter_context(tc.tile_pool(name="acc", bufs=1))
    psA_pool = ctx.enter_context(tc.tile_pool(name="psA", bufs=3, space="PSUM"))
    psT_pool = ctx.enter_context(tc.tile_pool(name="psT", bufs=3, space="PSUM"))
    ps2_pool = ctx.enter_context(tc.tile_pool(name="ps2", bufs=2, space="PSUM"))

    # constants: one packed blob, one HWDGE DMA (fewer exit-drain sem
    # waits and less Sync-queue startup time); tiles are views into it.
    nco = SHED_ROWS * 64
    blob = const_pool.tile([128, nco + 256 + 64 + 16 + 128], BF16)
    nc.sync.dma_start(blob[:], consts_ap)
    wa17 = blob[:, 0:nco].rearrange("p (r i) -> p r i", r=SHED_ROWS)
    wa4 = blob[:, nco:nco + 256].rearrange("p (r i) -> p r i", r=8)
    wa2 = blob[:, nco + 256:nco + 320].rearrange("p (r i) -> p r i", r=4)
    cwt = blob[:, nco + 320:nco + 336].rearrange("p (c j) -> p c j", c=2)
    ident = blob[:, nco + 336:nco + 464]

    acc = acc_pool.tile([8, 1], F32)
    nc.vector.memset(acc[:], 0.0)

    # issue ALL input DMAs upfront (SWDGE, f32->bf16 inline cast).
    subs = []
    for g in range(N_MAIN):
        gt = in8_pool.tile([128, SHED_ROWS, 256], BF16, tag="gt8")
        fl = delta_ap[GRP * g:GRP * g + GRP].rearrange("q h w -> (q h) w")
        nc.gpsimd.dma_start(
            gt[:],
            fl.rearrange("(p r) w -> p (r w)", p=128, r=SHED_ROWS),
        )
        subs.append((gt, "main" if g < N_MAIN - 1 else "tail", GRP))
    gt4 = in4_pool.tile([128, 8, 256], BF16, tag="gt4")
    src = delta_ap[88:92]
    nc.gpsimd.dma_start(
        gt4[:],
        src.rearrange("q (pp r) w -> (q pp) (r w)", pp=32, r=8),
    )
    subs.append((gt4, "tail", 4))
    for t in range(2):
        gt2 = in4_pool.tile([128, 4, 256], BF16, tag="gt2")
        src = delta_ap[92 + 2 * t:94 + 2 * t]
        nc.gpsimd.dma_start(
            gt2[:],
            src.rearrange("q (pp r) w -> (q pp) (r w)", pp=64, r=4),
        )
        subs.append((gt2, "tail", 2))

    def stage_a(sub):
        gt, kind, n_img = sub
        n_out = 8 * n_img
        psumA = psA_pool.tile([n_out, 256], F32, tag="psA")
        wA = {8: wa17, 4: wa4, 2: wa2}[n_img]
        rows = gt.shape[1]
        for r in range(rows):
            nc.tensor.matmul(
                psumA[:], lhsT=wA[:, r, :], rhs=gt[:, r, :],
                start=(r == 0), stop=(r == rows - 1),
            )
        # PSUM -> SBUF with f32->bf16 cast (ACT engine; off the PE
        # timeline).  Two half tiles so stage B's first transpose only
        # waits on the first half (shortens the post-stream tail).
        sAs = []
        for wc in range(2):
            sA = sS_pool.tile([n_out, 128], BF16, tag=f"sA{wc}")
            nc.scalar.copy(sA[:], psumA[:, 128 * wc:128 * wc + 128])
            sAs.append(sA)
        return sAs, n_out

    def stage_b_front(sAs, n_out, tS=None, off=0):
        # 2 PE transposes (own PSUM tiles: transpose-mode output must start
        # at a bank boundary on HW) + DVE copies out.  Tail groups copy into
        # free-axis offsets of a SHARED tS so one combined back-half covers
        # all of them (shorter post-stream serial chain).
        if tS is None:
            tS = tS_pool.tile([128, 2, n_out], BF16, tag="tS")
        for wc in range(2):
            tp = psT_pool.tile([128, n_out], BF16, tag="tp")
            nc.tensor.transpose(
                tp[:],
                sAs[wc][:],
                ident[0:n_out, 0:n_out],
            )
            nc.vector.tensor_copy(tS[:, wc, off:off + n_out], tp[:])
        return tS

    def stage_b_back(tS, n_out, accumulate=True):
        # contract w into ps2[j, (q,i)], fused |.|+sum, accumulate.
        ps2 = ps2_pool.tile([8, n_out], F32, tag="ps2")
        for wc in range(2):
            nc.tensor.matmul(
                ps2[:],
                lhsT=cwt[:, wc, :],
                rhs=tS[:, wc, 0:n_out],
                start=(wc == 0),
                stop=(wc == 1),
            )
        red = red_pool.tile([8, 1], F32)
        nc.vector.tensor_reduce(
            red[:], ps2[:], axis=mybir.AxisListType.X,
            op=mybir.AluOpType.add, apply_absolute_value=True,
        )
        if accumulate:
            nc.vector.tensor_add(acc[:], acc[:], red[:])
        return red

    # Software pipeline: emit stage B of group g-1 AFTER stage A of group g,
    # so the PE (which executes in program order) never stalls mid-stream on
    # the ACT/DVE round-trips of stage B.  The tails share one tS (free-axis
    # offsets 0/32/48) and ONE combined back-half.
    tSc = tS_pool.tile([128, 2, 128], BF16, tag="tSc")
    state = {"off": 0}

    def flush(prev):
        sAs, n_out, kind = prev
        if kind == "main":
            tS = stage_b_front(sAs, n_out)
            stage_b_back(tS, n_out)
        else:
            stage_b_front(sAs, n_out, tS=tSc, off=state["off"])
            state["off"] += n_out

    prev = None
    for sub in subs:
        sAs, n_out = stage_a(sub)
        cur = (sAs, n_out, sub[1])
        if prev is not None:
            flush(prev)
        prev = cur
    flush(prev)
    # acc holds the main groups; ship it while the tails finish, and ship
    # the tails' combined unaccumulated reduction separately so the final
    # out-DMA depends only on the last reduce (skips one DVE add).
    nc.sync.dma_start(out_ap[:, 0:1], acc[:])
    last_red = stage_b_back(tSc, 128, accumulate=False)
    nc.sync.dma_start(out_ap[:, 1:2], last_red[:])


# ---------------------------------------------------------------------------
# Build + run.
# ---------------------------------------------------------------------------

_CACHED_NC = None


def _build(for_sim=False):
    global _CACHED_NC, _USE_STOCK_TAIL
    if not for_sim and _CACHED_NC is not None:
        return _CACHED_NC
    _USE_STOCK_TAIL = for_sim
    nc = bass.Bass("TRN2", target_bir_lowering=False, debug=False)
    delta = nc.dram_tensor("delta", [IMGS_PER_CORE, H, W], F32, kind="ExternalInput")
    consts = nc.dram_tensor("consts", list(CONSTS.shape), BF16, kind="ExternalInput")
    out = nc.dram_tensor("out", [8, 2], F32, kind="ExternalOutput")

    with tile.TileContext(nc) as tc:
        _lowfreq_kernel(tc, out.ap(), delta.ap(), consts.ap())
    _USE_STOCK_TAIL = False
    if for_sim:
        return nc
    _strip_main_barrier(nc)
    _split_multi_waits(nc)
    _CACHED_NC = nc
    return nc


def _run(delta, **spmd_kwargs):
    import os
    os.environ["JAX_PLATFORMS"] = "axon"   # harness may have pinned cpu for the reference
    nc = _build()
    delta = np.ascontiguousarray(np.asarray(delta, dtype=np.float32))
    assert delta.shape == (B, C, H, W)
    shards = delta.reshape(N_CORES, IMGS_PER_CORE, H, W)
    in_maps = [
        {
            "delta": shards[i],
            "consts": CONSTS,
        }
        for i in range(N_CORES)
    ]
    try:
        res = bass_utils.run_bass_kernel_spmd(
            nc, in_maps, core_ids=list(range(N_CORES)), **spmd_kwargs
        )
    except Exception:
        # transient NRT_EXEC_UNIT_UNRECOVERABLE has been observed on this
        # terminal; one retry typically succeeds.
        res = bass_utils.run_bass_kernel_spmd(
            nc, in_maps, core_ids=list(range(N_CORES)), **spmd_kwargs
        )
    total = np.float64(0.0)
    for r in res.results:
        total += np.asarray(r["out"], np.float64).sum()
    return np.float32(total / TOTAL_LOW).reshape(()), res


def kernel(delta):
    out, _ = _run(delta)
    return out
